# revision 21
# baseline (speedup 1.0000x reference)
"""Trainium2 Bass kernel for Llama-style GQA attention prefill (S=2048).

Sharding: tensor-parallel over heads across 8 NeuronCores.
Each core owns 4 query heads + 1 KV head (GQA group-aligned), computes
its partial o_proj contribution (Wo input-sharded), and the host sums
the 8 partials.

Math notes:
- The paged-KV write+gather in the reference is an identity whenever
  page_indices are distinct (they are: arange(128)), so the kernel
  computes plain causal GQA attention with RoPE.
- Matmuls run in bf16 (4x faster than fp32 on the PE) with fp32 PSUM
  accumulation. Score scale 1/sqrt(D) is folded into Wq on the host.
- Attention uses a transposed-score layout: scoresT[k, q] so softmax
  needs no PE transposes. exp() is taken without max subtraction
  (scores are O(10), safe in fp32). The denominator is computed with an
  all-ones [128,128] stationary matmul accumulated alongside the PV
  matmul: its PSUM result is the softmax denominator already broadcast
  to all 128 partitions (same PE cost as an M=1 ones-vector matmul,
  since matmul time scales with N only). Normalization is then just a
  single-pass approximate reciprocal (VectorE, ~18-bit) and a multiply,
  entirely off the PE critical path.
- o_proj is fused into the attention loop (per 512-row query block) so
  its matmuls and output DMAs overlap attention instead of forming a
  serial tail. Output partials are written in bf16.
"""

import sys

if "/opt/trn_rl_repo" not in sys.path:
    sys.path.insert(0, "/opt/trn_rl_repo")

import numpy as np
import ml_dtypes

BF = ml_dtypes.bfloat16

S = 2048
HID = 4096
D = 128
H = 32
HKV = 8
NCORES = 8
NQ = H // NCORES  # 4 query heads per core
ROPE_THETA = 10000.0

_NC_CACHE = {}


def build_nc(s=S, hid=HID, nq=NQ):
    """Build the per-core Bass program (same program for all 8 cores)."""
    import concourse.bass as bass
    import concourse.mybir as mybir
    import concourse.tile as tile
    from concourse import bacc
    from concourse.masks import make_identity

    f32 = mybir.dt.float32
    bf16 = mybir.dt.bfloat16
    Exp = mybir.ActivationFunctionType.Exp

    KB = hid // 128   # hidden contraction blocks
    SBn = s // 512    # 512-wide sequence blocks
    KTn = s // 128    # 128-wide key tiles
    NDB = nq + 2      # projection d-blocks: k, v, q0..q{nq-1}
    WC = NDB * 128    # wqkvT columns
    HB = hid // 512   # output hidden blocks

    nc = bacc.Bacc("TRN2")

    xT_d = nc.dram_tensor("xT", [hid, s], bf16, kind="ExternalInput")
    w_d = nc.dram_tensor("wqkvT", [hid, WC], bf16, kind="ExternalInput")
    wo_d = nc.dram_tensor("woT", [nq * 128, hid], bf16, kind="ExternalInput")
    cos2_d = nc.dram_tensor("cos2", [128, s], f32, kind="ExternalInput")
    sin2_d = nc.dram_tensor("sin2", [128, s], f32, kind="ExternalInput")
    tri_d = nc.dram_tensor("tri", [128, 128], bf16, kind="ExternalInput")
    swpm_d = nc.dram_tensor("swpm", [128, 128], f32, kind="ExternalInput")
    out_d = nc.dram_tensor("out", [s, hid], bf16, kind="ExternalOutput")

    with tile.TileContext(nc) as tc:
        with (
            tc.tile_pool(name="const", bufs=1) as const_pool,
            tc.tile_pool(name="qkv", bufs=1) as qkv_pool,
        ):
            tri = const_pool.tile([128, 128], bf16, tag="tri")
            onesJ = const_pool.tile([128, 128], bf16, tag="onesJ")
            ident = const_pool.tile([128, 128], bf16, tag="ident")
            swpm = const_pool.tile([128, 128], f32, tag="swpm")

            # persistent per-head tensors
            qk = [
                qkv_pool.tile([128, s], bf16, tag=f"qk{i}", name=f"qk{i}")
                for i in range(nq + 1)
            ]  # qk[0..nq-1] = q heads (T layout [d, s]); qk[nq] = kT
            vt = qkv_pool.tile([128, KTn, 128], bf16, tag="vt")  # v natural [s,d] tiles

            # ---------------- Phase 1: projections + RoPE ----------------
            # Weights live in 32 per-kb tiles so the first matmuls only wait
            # for the first small DMAs; DMA issue order is interleaved
            # (w[kb], x[kb]) so PE starts ~1us in.
            with (
                tc.tile_pool(name="cs", bufs=1) as cs_pool,
                tc.tile_pool(name="xt", bufs=2) as xt_pool,
                tc.tile_pool(name="wsb", bufs=1) as w_pool,
                tc.tile_pool(name="pp", bufs=4, space="PSUM") as pp,
                tc.tile_pool(name="tpp", bufs=2, space="PSUM") as tpp,
                tc.tile_pool(name="spp", bufs=2, space="PSUM") as spp,
                tc.tile_pool(name="rtmp", bufs=3) as rt,
                tc.tile_pool(name="vstage", bufs=2) as vs,
            ):
                cos2 = cs_pool.tile([128, s], f32, tag="cos2")
                sin2 = cs_pool.tile([128, s], f32, tag="sin2")
                w_tiles = [
                    w_pool.tile([128, WC], bf16, tag=f"w{kb}", name=f"w{kb}")
                    for kb in range(KB)
                ]
                wv_view = w_d[:, :].rearrange("(t p) c -> p t c", p=128)

                # DMA order: first kb's weights + x tile, then small consts,
                # then the remaining kb's interleaved.
                first_xts = []
                nc.sync.dma_start(w_tiles[0], wv_view[:, 0, :])
                xtile0 = xt_pool.tile([128, 512], bf16, tag="xt0", name="xt0")
                nc.sync.dma_start(xtile0, xT_d[0:128, 0:512])
                first_xts.append(xtile0)
                nc.sync.dma_start(swpm, swpm_d[:, :])
                nc.vector.memset(onesJ, 1.0)
                make_identity(nc, ident)
                for kb in range(1, KB):
                    nc.sync.dma_start(w_tiles[kb], wv_view[:, kb, :])
                    xtile = xt_pool.tile(
                        [128, 512], bf16, tag=f"xt{kb}", name=f"xt{kb}"
                    )
                    nc.sync.dma_start(xtile, xT_d[kb * 128 : (kb + 1) * 128, 0:512])
                    first_xts.append(xtile)
                    if kb == 2:
                        # cos/sin for the first seq block only — the full
                        # 2MB here would stall the weight/x stream ~5us
                        nc.sync.dma_start(cos2[:, 0:512], cos2_d[:, 0:512])
                        nc.sync.dma_start(sin2[:, 0:512], sin2_d[:, 0:512])
                    elif kb == 6:
                        nc.sync.dma_start(tri, tri_d[:, :])
                nc.sync.dma_start(cos2[:, 512:s], cos2_d[:, 512:s])
                nc.sync.dma_start(sin2[:, 512:s], sin2_d[:, 512:s])

                # d-block order: k(0), v(1), then q heads (2..)
                for sb in range(SBn):
                    sl = slice(sb * 512, (sb + 1) * 512)
                    if sb == 0:
                        xts = first_xts
                    else:
                        xts = []
                        for kb in range(KB):
                            xtile = xt_pool.tile(
                                [128, 512], bf16, tag=f"xt{kb}", name=f"xt{kb}"
                            )
                            nc.sync.dma_start(
                                xtile, xT_d[kb * 128 : (kb + 1) * 128, sl]
                            )
                            xts.append(xtile)
                    for db in range(NDB):
                        ps = pp.tile([128, 512], f32, tag="pp")
                        for kb in range(KB):
                            nc.tensor.matmul(
                                ps,
                                w_tiles[kb][:, db * 128 : (db + 1) * 128],
                                xts[kb],
                                start=(kb == 0),
                                stop=(kb == KB - 1),
                            )
                        if db == 1:
                            # v: cast to bf16 then transpose to natural [s, d]
                            vstg = vs.tile([128, 512], bf16, tag="vstg")
                            nc.scalar.copy(vstg, ps)
                            for j in range(4):
                                tps = tpp.tile([128, 128], bf16, tag="tpp")
                                nc.tensor.transpose(
                                    tps, vstg[:, j * 128 : (j + 1) * 128], ident
                                )
                                nc.scalar.copy(vt[:, sb * 4 + j, :], tps)
                        else:
                            # RoPE: dst = p * COS2 + swap(p) * SIN2
                            # (half-swap via PE with a constant permutation)
                            dst = qk[nq] if db == 0 else qk[db - 2]
                            pcp = rt.tile([128, 512], f32, tag="pcp")
                            nc.scalar.copy(pcp, ps)
                            sps = spp.tile([128, 512], f32, tag="sps")
                            nc.tensor.matmul(sps, swpm, pcp, start=True, stop=True)
                            nc.vector.tensor_mul(pcp, pcp, cos2[:, sl])
                            swp = rt.tile([128, 512], f32, tag="swp")
                            nc.vector.tensor_mul(swp, sps, sin2[:, sl])
                            nc.vector.tensor_add(dst[:, sl], pcp, swp)

            # ---------- Phase 2+3: attention fused with o_proj ----------
            kT = qk[nq]
            with tc.tile_pool(name="wosb", bufs=1) as wo_pool:
                wo_sb = wo_pool.tile([128, nq, hid], bf16, tag="wosb")
                nc.sync.dma_start(
                    wo_sb, wo_d[:, :].rearrange("(t p) c -> p t c", p=128)
                )
                with (
                    tc.tile_pool(name="scp", bufs=2, space="PSUM") as scp,
                    tc.tile_pool(name="atp", bufs=2, space="PSUM") as atp,
                    tc.tile_pool(name="bsp", bufs=2, space="PSUM") as bsp,
                    tc.tile_pool(name="outp", bufs=2, space="PSUM") as outp,
                    tc.tile_pool(name="exps", bufs=6) as exps,
                    tc.tile_pool(name="dsm", bufs=2) as dsm,
                    tc.tile_pool(name="atsb", bufs=2) as atsb,
                    tc.tile_pool(name="osb", bufs=6) as osb,
                ):
                    for qb in range(SBn):
                        at_qb = []
                        for h in range(nq):
                            acc = atp.tile([128, 512], f32, tag="acc")
                            bsum = bsp.tile([128, 512], f32, tag="bsum")
                            nkt = 4 * qb + 4
                            for kt in range(nkt):
                                jstart = max(0, 128 * (kt - 4 * qb))
                                w = 512 - jstart
                                q_lo = qb * 512 + jstart
                                sc = scp.tile([128, 512], f32, tag="sc")
                                nc.tensor.matmul(
                                    sc[:, :w],
                                    kT[:, kt * 128 : (kt + 1) * 128],
                                    qk[h][:, q_lo : (qb + 1) * 512],
                                    start=True,
                                    stop=True,
                                )
                                ex = exps.tile([128, 512], bf16, tag="ex")
                                nc.scalar.activation(ex[:, :w], sc[:, :w], Exp)
                                if kt >= 4 * qb:
                                    nc.vector.tensor_mul(
                                        ex[:, 0:128], ex[:, 0:128], tri
                                    )
                                # denominator, pre-broadcast to all 128
                                # partitions via all-ones stationary operand
                                nc.tensor.matmul(
                                    bsum[:, jstart:512],
                                    onesJ,
                                    ex[:, :w],
                                    start=(kt == 0),
                                    stop=(kt == nkt - 1),
                                )
                                nc.tensor.matmul(
                                    acc[:, jstart:512],
                                    vt[:, kt, :],
                                    ex[:, :w],
                                    start=(kt == 0),
                                    stop=(kt == nkt - 1),
                                )
                            # normalization fully off the PE: approx
                            # reciprocal (~18 bits, plenty at 2e-2 tol)
                            # then scale.
                            rb = dsm.tile([128, 512], f32, tag="rb")
                            nc.vector.reciprocal_approx_fast(rb, bsum)
                            ath = atsb.tile(
                                [128, 512], bf16, tag=f"at{h}", name=f"at{h}"
                            )
                            nc.vector.tensor_mul(ath, acc, rb)
                            at_qb.append(ath)

                        # o_proj for this query block (contraction over heads)
                        for sti in range(4):
                            ssl = slice(sti * 128, (sti + 1) * 128)
                            orow = slice(qb * 512 + sti * 128, qb * 512 + (sti + 1) * 128)
                            for nb in range(HB):
                                nsl = slice(nb * 512, (nb + 1) * 512)
                                po = outp.tile([128, 512], f32, tag="po")
                                for h in range(nq):
                                    nc.tensor.matmul(
                                        po,
                                        at_qb[h][:, ssl],
                                        wo_sb[:, h, nsl],
                                        start=(h == 0),
                                        stop=(h == nq - 1),
                                    )
                                ot = osb.tile([128, 512], bf16, tag="ot")
                                nc.vector.tensor_copy(ot, po)
                                nc.sync.dma_start(out_d[orow, nsl], ot)

    nc.compile()
    nc.finalize()
    return nc


def _prep_core_inputs(x_np, position_ids, Wq, Wk, Wv, Wo):
    """Host-side sharding/layout prep. Returns list of per-core input dicts."""
    scale = float(D) ** -0.5
    xT = np.ascontiguousarray(x_np.T).astype(BF)

    pos = np.asarray(position_ids).astype(np.float32)
    half = D // 2
    inv_freq = 1.0 / (ROPE_THETA ** (np.arange(half, dtype=np.float32) / half))
    ang = pos[:, None] * inv_freq[None, :]  # [S, 64]
    cosT = np.cos(ang).T.astype(np.float32)  # [64, S]
    sinT = np.sin(ang).T.astype(np.float32)
    cos2 = np.concatenate([cosT, cosT], axis=0)  # [128, S]
    sin2 = np.concatenate([-sinT, sinT], axis=0)  # [128, S]
    cos2 = np.ascontiguousarray(cos2)
    sin2 = np.ascontiguousarray(sin2)

    tri = np.triu(np.ones((128, 128), np.float32)).astype(BF)  # [k, q]: q >= k
    swpm = np.zeros((128, 128), np.float32)
    swpm[np.arange(128), (np.arange(128) + 64) % 128] = 1.0  # half-swap perm

    Wq_s = (np.asarray(Wq, np.float32) * scale)
    Wk = np.asarray(Wk, np.float32)
    Wv = np.asarray(Wv, np.float32)
    Wo = np.asarray(Wo, np.float32)

    in_maps = []
    for c in range(NCORES):
        qrows = Wq_s[c * NQ * D : (c + 1) * NQ * D]  # [512, HID]
        krows = Wk[c * D : (c + 1) * D]  # [128, HID]
        vrows = Wv[c * D : (c + 1) * D]
        # column order in wqkvT: k, v, q0..q3
        wqkv = np.concatenate([krows, vrows, qrows], axis=0)  # [768, HID]
        wqkvT = np.ascontiguousarray(wqkv.T).astype(BF)  # [HID, 768]
        woT = np.ascontiguousarray(Wo[:, c * NQ * D : (c + 1) * NQ * D].T).astype(
            BF
        )  # [512, HID]
        in_maps.append(
            {
                "xT": xT,
                "wqkvT": wqkvT,
                "woT": woT,
                "cos2": cos2,
                "sin2": sin2,
                "tri": tri,
                "swpm": swpm,
            }
        )
    return in_maps


def kernel(
    hidden_states,
    position_ids,
    page_indices,
    Wq,
    Wk,
    Wv,
    Wo,
    kv_cache,
    _trace=False,
):
    from concourse.bass_utils import run_bass_kernel_spmd

    x = np.asarray(hidden_states, np.float32)[0]  # [S, HID]
    pidx = np.asarray(page_indices)
    # write-then-gather through distinct pages is the identity
    assert len(np.unique(pidx)) == pidx.shape[0], "page_indices must be distinct"

    in_maps = _prep_core_inputs(x, position_ids, Wq, Wk, Wv, Wo)

    if "nc" not in _NC_CACHE:
        _NC_CACHE["nc"] = build_nc()
    nc = _NC_CACHE["nc"]

    res = run_bass_kernel_spmd(
        nc, in_maps, core_ids=list(range(NCORES)), trace=_trace
    )
    out = np.zeros((S, HID), np.float32)
    for c in range(NCORES):
        out += np.asarray(res.results[c]["out"], np.float32)
    if _trace:
        kernel.last_results = res
    return out[None].astype(np.float32)


# revision 31
# speedup vs baseline: 1.0286x; 1.0286x over previous
"""Trainium2 Bass kernel for Llama-style GQA attention prefill (S=2048).

Sharding: tensor-parallel over heads across 8 NeuronCores.
Each core owns 4 query heads + 1 KV head (GQA group-aligned), computes
its partial o_proj contribution (Wo input-sharded), and the host sums
the 8 partials.

Math notes:
- The paged-KV write+gather in the reference is an identity whenever
  page_indices are distinct (they are: arange(128)), so the kernel
  computes plain causal GQA attention with RoPE.
- Matmuls run in bf16 (4x faster than fp32 on the PE) with fp32 PSUM
  accumulation. Score scale 1/sqrt(D) is folded into Wq on the host.
- Attention uses a transposed-score layout: scoresT[k, q] so softmax
  needs no PE transposes. exp() is taken without max subtraction
  (scores are O(10), safe in fp32). The denominator is computed with an
  all-ones [128,128] stationary matmul accumulated alongside the PV
  matmul: its PSUM result is the softmax denominator already broadcast
  to all 128 partitions (same PE cost as an M=1 ones-vector matmul,
  since matmul time scales with N only). Normalization is then just a
  single-pass approximate reciprocal (VectorE, ~18-bit) and a multiply,
  entirely off the PE critical path.
- o_proj is fused into the attention loop (per 512-row query block) so
  its matmuls and output DMAs overlap attention instead of forming a
  serial tail. Output partials are written in bf16.
"""

import sys

if "/opt/trn_rl_repo" not in sys.path:
    sys.path.insert(0, "/opt/trn_rl_repo")

import numpy as np
import ml_dtypes

BF = ml_dtypes.bfloat16

S = 2048
HID = 4096
D = 128
H = 32
HKV = 8
NCORES = 8
NQ = H // NCORES  # 4 query heads per core
ROPE_THETA = 10000.0

_NC_CACHE = {}


def build_nc(s=S, hid=HID, nq=NQ):
    """Build the per-core Bass program (same program for all 8 cores)."""
    import concourse.bass as bass
    import concourse.mybir as mybir
    import concourse.tile as tile
    from concourse import bacc
    from concourse.masks import make_identity

    f32 = mybir.dt.float32
    bf16 = mybir.dt.bfloat16
    Exp = mybir.ActivationFunctionType.Exp

    KB = hid // 128   # hidden contraction blocks
    SBn = s // 512    # 512-wide sequence blocks
    KTn = s // 128    # 128-wide key tiles
    NDB = nq + 2      # projection d-blocks: k, v, q0..q{nq-1}
    WC = NDB * 128    # wqkvT columns
    HB = hid // 512   # output hidden blocks

    nc = bacc.Bacc("TRN2")

    xT_d = nc.dram_tensor("xT", [hid, s], bf16, kind="ExternalInput")
    w_d = nc.dram_tensor("wqkvT", [hid, WC], bf16, kind="ExternalInput")
    wo_d = nc.dram_tensor("woT", [nq * 128, hid], bf16, kind="ExternalInput")
    cos2_d = nc.dram_tensor("cos2", [128, s], f32, kind="ExternalInput")
    sin2_d = nc.dram_tensor("sin2", [128, s], f32, kind="ExternalInput")
    tri_d = nc.dram_tensor("tri", [128, 128], bf16, kind="ExternalInput")
    swpm_d = nc.dram_tensor("swpm", [128, 128], f32, kind="ExternalInput")
    out_d = nc.dram_tensor("out", [s, hid], bf16, kind="ExternalOutput")

    with tile.TileContext(nc) as tc:
        with (
            tc.tile_pool(name="const", bufs=1) as const_pool,
            tc.tile_pool(name="qkv", bufs=1) as qkv_pool,
        ):
            tri = const_pool.tile([128, 128], bf16, tag="tri")
            onesJ = const_pool.tile([128, 128], bf16, tag="onesJ")
            ident = const_pool.tile([128, 128], bf16, tag="ident")
            swpm = const_pool.tile([128, 128], f32, tag="swpm")

            # persistent per-head tensors
            qk = [
                qkv_pool.tile([128, s], bf16, tag=f"qk{i}", name=f"qk{i}")
                for i in range(nq + 1)
            ]  # qk[0..nq-1] = q heads (T layout [d, s]); qk[nq] = kT
            vt = qkv_pool.tile([128, KTn, 128], bf16, tag="vt")  # v natural [s,d] tiles

            # ---------------- Phase 1: projections + RoPE ----------------
            # Weights live in 32 per-kb tiles so the first matmuls only wait
            # for the first small DMAs; DMA issue order is interleaved
            # (w[kb], x[kb]) so PE starts ~1us in.
            with (
                tc.tile_pool(name="cs", bufs=1) as cs_pool,
                tc.tile_pool(name="xt", bufs=2) as xt_pool,
                tc.tile_pool(name="wsb", bufs=1) as w_pool,
                tc.tile_pool(name="pp", bufs=4, space="PSUM") as pp,
                tc.tile_pool(name="tpp", bufs=2, space="PSUM") as tpp,
                tc.tile_pool(name="spp", bufs=2, space="PSUM") as spp,
                tc.tile_pool(name="rtmp", bufs=3) as rt,
                tc.tile_pool(name="vstage", bufs=2) as vs,
            ):
                cos2 = cs_pool.tile([128, s], f32, tag="cos2")
                sin2 = cs_pool.tile([128, s], f32, tag="sin2")
                w_tiles = [
                    w_pool.tile([128, WC], bf16, tag=f"w{kb}", name=f"w{kb}")
                    for kb in range(KB)
                ]
                wv_view = w_d[:, :].rearrange("(t p) c -> p t c", p=128)

                # DMA order: first kb's weights + x tile, then small consts,
                # then the remaining kb's interleaved.
                first_xts = []
                nc.sync.dma_start(w_tiles[0], wv_view[:, 0, :])
                xtile0 = xt_pool.tile([128, 512], bf16, tag="xt0", name="xt0")
                nc.sync.dma_start(xtile0, xT_d[0:128, 0:512])
                first_xts.append(xtile0)
                nc.sync.dma_start(swpm, swpm_d[:, :])
                nc.vector.memset(onesJ, 1.0)
                make_identity(nc, ident)
                for kb in range(1, KB):
                    nc.sync.dma_start(w_tiles[kb], wv_view[:, kb, :])
                    xtile = xt_pool.tile(
                        [128, 512], bf16, tag=f"xt{kb}", name=f"xt{kb}"
                    )
                    nc.sync.dma_start(xtile, xT_d[kb * 128 : (kb + 1) * 128, 0:512])
                    first_xts.append(xtile)
                # cos/sin/tri after the weight/x stream: first needed at
                # ~35us (sb=0 RoPE), and injecting them earlier stalls the
                # DMA-paced projection start.
                nc.sync.dma_start(cos2[:, 0:512], cos2_d[:, 0:512])
                nc.sync.dma_start(sin2[:, 0:512], sin2_d[:, 0:512])
                nc.sync.dma_start(tri, tri_d[:, :])
                nc.sync.dma_start(cos2[:, 512:s], cos2_d[:, 512:s])
                nc.sync.dma_start(sin2[:, 512:s], sin2_d[:, 512:s])

                def finish_db(sb, db, ps):
                    sl = slice(sb * 512, (sb + 1) * 512)
                    if db == 1:
                        # v: cast to bf16 then transpose to natural [s, d]
                        vstg = vs.tile([128, 512], bf16, tag="vstg")
                        nc.scalar.copy(vstg, ps)
                        for j in range(4):
                            tps = tpp.tile([128, 128], bf16, tag="tpp")
                            nc.tensor.transpose(
                                tps, vstg[:, j * 128 : (j + 1) * 128], ident
                            )
                            nc.scalar.copy(vt[:, sb * 4 + j, :], tps)
                    else:
                        # RoPE: dst = p * COS2 + swap(p) * SIN2
                        # (half-swap via PE with a constant permutation)
                        dst = qk[nq] if db == 0 else qk[db - 2]
                        pcp = rt.tile([128, 512], f32, tag="pcp")
                        nc.scalar.copy(pcp, ps)
                        sps = spp.tile([128, 512], f32, tag="sps")
                        nc.tensor.matmul(sps, swpm, pcp, start=True, stop=True)
                        nc.vector.tensor_mul(pcp, pcp, cos2[:, sl])
                        swp = rt.tile([128, 512], f32, tag="swp")
                        nc.vector.tensor_mul(swp, sps, sin2[:, sl])
                        nc.vector.tensor_add(dst[:, sl], pcp, swp)

                # sb=0: kb-major across 4 concurrently-open PSUM groups so
                # the PE gets 4 matmuls of work per arriving (w, x) DMA pair
                # instead of 1 (the startup is DMA-bandwidth-paced).
                ps4 = [
                    pp.tile([128, 512], f32, tag="pp", name=f"ps4_{i}")
                    for i in range(4)
                ]
                for kb in range(KB):
                    for db in range(4):
                        nc.tensor.matmul(
                            ps4[db],
                            w_tiles[kb][:, db * 128 : (db + 1) * 128],
                            first_xts[kb],
                            start=(kb == 0),
                            stop=(kb == KB - 1),
                        )
                for db in range(4):
                    finish_db(0, db, ps4[db])
                for db in range(4, NDB):
                    ps = pp.tile([128, 512], f32, tag="pp")
                    for kb in range(KB):
                        nc.tensor.matmul(
                            ps,
                            w_tiles[kb][:, db * 128 : (db + 1) * 128],
                            first_xts[kb],
                            start=(kb == 0),
                            stop=(kb == KB - 1),
                        )
                    finish_db(0, db, ps)

                # sb>=1: x is prefetched during the previous block's compute,
                # so the simple db-major order is already PE-bound.
                for sb in range(1, SBn):
                    sl = slice(sb * 512, (sb + 1) * 512)
                    xts = []
                    for kb in range(KB):
                        xtile = xt_pool.tile(
                            [128, 512], bf16, tag=f"xt{kb}", name=f"xt{kb}"
                        )
                        nc.sync.dma_start(
                            xtile, xT_d[kb * 128 : (kb + 1) * 128, sl]
                        )
                        xts.append(xtile)
                    for db in range(NDB):
                        ps = pp.tile([128, 512], f32, tag="pp")
                        for kb in range(KB):
                            nc.tensor.matmul(
                                ps,
                                w_tiles[kb][:, db * 128 : (db + 1) * 128],
                                xts[kb],
                                start=(kb == 0),
                                stop=(kb == KB - 1),
                            )
                        finish_db(sb, db, ps)

            # ---------- Phase 2+3: attention fused with o_proj ----------
            kT = qk[nq]
            with tc.tile_pool(name="wosb", bufs=1) as wo_pool:
                wo_sb = wo_pool.tile([128, nq, hid], bf16, tag="wosb")
                nc.sync.dma_start(
                    wo_sb, wo_d[:, :].rearrange("(t p) c -> p t c", p=128)
                )
                with (
                    tc.tile_pool(name="scp", bufs=2, space="PSUM") as scp,
                    tc.tile_pool(name="atp", bufs=2, space="PSUM") as atp,
                    tc.tile_pool(name="bsp", bufs=2, space="PSUM") as bsp,
                    tc.tile_pool(name="outp", bufs=2, space="PSUM") as outp,
                    tc.tile_pool(name="exps", bufs=6) as exps,
                    tc.tile_pool(name="dsm", bufs=4) as dsm,
                    tc.tile_pool(name="atsb", bufs=2) as atsb,
                    tc.tile_pool(name="osb", bufs=6) as osb,
                ):
                    for qb in range(SBn):
                        at_qb = []
                        for h in range(nq):
                            acc = atp.tile([128, 512], f32, tag="acc")
                            bsum = bsp.tile([128, 512], f32, tag="bsum")
                            nkt = 4 * qb + 4
                            for ki, kt in enumerate(range(nkt)):
                                jstart = max(0, 128 * (kt - 4 * qb))
                                w = 512 - jstart
                                q_lo = qb * 512 + jstart
                                sc = scp.tile([128, 512], f32, tag="sc")
                                nc.tensor.matmul(
                                    sc[:, :w],
                                    kT[:, kt * 128 : (kt + 1) * 128],
                                    qk[h][:, q_lo : (qb + 1) * 512],
                                    start=True,
                                    stop=True,
                                )
                                ex = exps.tile([128, 512], bf16, tag="ex")
                                nc.scalar.activation(ex[:, :w], sc[:, :w], Exp)
                                if kt >= 4 * qb:
                                    nc.vector.tensor_mul(
                                        ex[:, 0:128], ex[:, 0:128], tri
                                    )
                                # denominator, pre-broadcast to all 128
                                # partitions via all-ones stationary operand
                                nc.tensor.matmul(
                                    bsum[:, jstart:512],
                                    onesJ,
                                    ex[:, :w],
                                    start=(ki == 0),
                                    stop=(ki == nkt - 1),
                                )
                                nc.tensor.matmul(
                                    acc[:, jstart:512],
                                    vt[:, kt, :],
                                    ex[:, :w],
                                    start=(ki == 0),
                                    stop=(ki == nkt - 1),
                                )
                            # normalization fully off the PE: approx
                            # reciprocal (~18 bits, plenty at 2e-2 tol)
                            # then scale.
                            rb = dsm.tile([128, 512], f32, tag="rb")
                            nc.vector.reciprocal_approx_fast(rb, bsum)
                            ath = atsb.tile(
                                [128, 512], bf16, tag=f"at{h}", name=f"at{h}"
                            )
                            nc.vector.tensor_mul(ath, acc, rb)
                            at_qb.append(ath)

                        # o_proj for this query block (contraction over heads)
                        for sti in range(4):
                            ssl = slice(sti * 128, (sti + 1) * 128)
                            orow = slice(qb * 512 + sti * 128, qb * 512 + (sti + 1) * 128)
                            for nb in range(HB):
                                nsl = slice(nb * 512, (nb + 1) * 512)
                                po = outp.tile([128, 512], f32, tag="po")
                                for h in range(nq):
                                    nc.tensor.matmul(
                                        po,
                                        at_qb[h][:, ssl],
                                        wo_sb[:, h, nsl],
                                        start=(h == 0),
                                        stop=(h == nq - 1),
                                    )
                                ot = osb.tile([128, 512], bf16, tag="ot")
                                nc.vector.tensor_copy(ot, po)
                                nc.sync.dma_start(out_d[orow, nsl], ot)

    nc.compile()
    nc.finalize()
    return nc


def _prep_core_inputs(x_np, position_ids, Wq, Wk, Wv, Wo):
    """Host-side sharding/layout prep. Returns list of per-core input dicts."""
    scale = float(D) ** -0.5
    xT = np.ascontiguousarray(x_np.T).astype(BF)

    pos = np.asarray(position_ids).astype(np.float32)
    half = D // 2
    inv_freq = 1.0 / (ROPE_THETA ** (np.arange(half, dtype=np.float32) / half))
    ang = pos[:, None] * inv_freq[None, :]  # [S, 64]
    cosT = np.cos(ang).T.astype(np.float32)  # [64, S]
    sinT = np.sin(ang).T.astype(np.float32)
    cos2 = np.concatenate([cosT, cosT], axis=0)  # [128, S]
    sin2 = np.concatenate([-sinT, sinT], axis=0)  # [128, S]
    cos2 = np.ascontiguousarray(cos2)
    sin2 = np.ascontiguousarray(sin2)

    tri = np.triu(np.ones((128, 128), np.float32)).astype(BF)  # [k, q]: q >= k
    swpm = np.zeros((128, 128), np.float32)
    swpm[np.arange(128), (np.arange(128) + 64) % 128] = 1.0  # half-swap perm

    Wq_s = (np.asarray(Wq, np.float32) * scale)
    Wk = np.asarray(Wk, np.float32)
    Wv = np.asarray(Wv, np.float32)
    Wo = np.asarray(Wo, np.float32)

    in_maps = []
    for c in range(NCORES):
        qrows = Wq_s[c * NQ * D : (c + 1) * NQ * D]  # [512, HID]
        krows = Wk[c * D : (c + 1) * D]  # [128, HID]
        vrows = Wv[c * D : (c + 1) * D]
        # column order in wqkvT: k, v, q0..q3
        wqkv = np.concatenate([krows, vrows, qrows], axis=0)  # [768, HID]
        wqkvT = np.ascontiguousarray(wqkv.T).astype(BF)  # [HID, 768]
        woT = np.ascontiguousarray(Wo[:, c * NQ * D : (c + 1) * NQ * D].T).astype(
            BF
        )  # [512, HID]
        in_maps.append(
            {
                "xT": xT,
                "wqkvT": wqkvT,
                "woT": woT,
                "cos2": cos2,
                "sin2": sin2,
                "tri": tri,
                "swpm": swpm,
            }
        )
    return in_maps


def kernel(
    hidden_states,
    position_ids,
    page_indices,
    Wq,
    Wk,
    Wv,
    Wo,
    kv_cache,
    _trace=False,
):
    from concourse.bass_utils import run_bass_kernel_spmd

    x = np.asarray(hidden_states, np.float32)[0]  # [S, HID]
    pidx = np.asarray(page_indices)
    # write-then-gather through distinct pages is the identity
    assert len(np.unique(pidx)) == pidx.shape[0], "page_indices must be distinct"

    in_maps = _prep_core_inputs(x, position_ids, Wq, Wk, Wv, Wo)

    if "nc" not in _NC_CACHE:
        _NC_CACHE["nc"] = build_nc()
    nc = _NC_CACHE["nc"]

    res = run_bass_kernel_spmd(
        nc, in_maps, core_ids=list(range(NCORES)), trace=_trace
    )
    out = np.zeros((S, HID), np.float32)
    for c in range(NCORES):
        out += np.asarray(res.results[c]["out"], np.float32)
    if _trace:
        kernel.last_results = res
    return out[None].astype(np.float32)
